# revision 2
# baseline (speedup 1.0000x reference)
"""GridNetwork RNN kernel for 8 Trainium2 NeuronCores.

Strategy: tensor-shard W_hh by output rows (512 per core), full batch on
every core, per-step AllGather of the hidden state in bf16. W_hh stays
SBUF-resident (bf16, 4MB/core). The decoder for step t-1 runs while the
AllGather for step t is in flight, hiding collective latency behind PE
work. The velocity input projection is folded into each step's PSUM
accumulation as an extra K=2 matmul, so vx is never materialized.

Layouts are transposed ([feature, batch]) so the batch (200) is the
matmul free dim and features map to partitions/contraction.
"""
import sys
sys.path.insert(0, '/opt/trn_rl_repo')
import numpy as np
import ml_dtypes

import concourse.bass as bass
import concourse.bacc as bacc
import concourse.tile as tile
import concourse.mybir as mybir
from concourse.bass_utils import run_bass_kernel_spmd

SEQ, B, NP, NG = 100, 200, 512, 4096
N_CORES = 8
GSL = NG // N_CORES      # 512 g-rows per core
PSL = NP // N_CORES      # 64 logit rows per core
KT = NG // 128           # 32 K tiles over the hidden dim
MT = GSL // 128          # 4 M tiles per core's W_hh slice

BF = mybir.dt.bfloat16
F32 = mybir.dt.float32


def build_nc(seq=SEQ, repeat=1):
    nc = bacc.Bacc("TRN2", target_bir_lowering=False, debug=False,
                   num_devices=N_CORES)
    whh = nc.dram_tensor("whh", [NG, GSL], BF, kind="ExternalInput").ap()
    wenc = nc.dram_tensor("wenc", [NP, GSL], BF, kind="ExternalInput").ap()
    wdec = nc.dram_tensor("wdec", [NG, PSL], BF, kind="ExternalInput").ap()
    wih = nc.dram_tensor("wih", [2, GSL], BF, kind="ExternalInput").ap()
    pct = nc.dram_tensor("pct", [NP, B], BF, kind="ExternalInput").ap()
    velt = nc.dram_tensor("velt", [2, seq * B], BF, kind="ExternalInput").ap()
    g_out = nc.dram_tensor("g_out", [seq, GSL, B], F32, kind="ExternalOutput").ap()
    l_out = nc.dram_tensor("l_out", [seq, PSL, B], F32, kind="ExternalOutput").ap()

    with tile.TileContext(nc) as tc:
        with (
            tc.tile_pool(name="const", bufs=1) as cpool,
            tc.tile_pool(name="hbuf", bufs=3) as hpool,
            tc.tile_pool(name="gbuf", bufs=8) as gpool,
            tc.tile_pool(name="hbf", bufs=8) as hbfpool,
            tc.tile_pool(name="lsb", bufs=4) as lpool,
            tc.tile_pool(name="ps", bufs=5, space="PSUM") as ps,
            tc.tile_pool(name="psd", bufs=2, space="PSUM") as psd,
            tc.tile_pool(name="dram", bufs=3, space="DRAM") as dram,
        ):
            # resident weights
            whh_sb = cpool.tile([128, KT * GSL], BF)       # [:, k*512+m*128 ..]
            for k in range(KT):
                nc.sync.dma_start(out=whh_sb[:, k * GSL:(k + 1) * GSL],
                                  in_=whh[k * 128:(k + 1) * 128, :])
            wdec_sb = cpool.tile([128, KT * PSL], BF)
            for k in range(KT):
                nc.sync.dma_start(out=wdec_sb[:, k * PSL:(k + 1) * PSL],
                                  in_=wdec[k * 128:(k + 1) * 128, :])
            wih_sb = cpool.tile([2, GSL], BF)
            nc.sync.dma_start(out=wih_sb[:], in_=wih)
            velt_sb = cpool.tile([2, seq * B], BF)
            nc.sync.dma_start(out=velt_sb[:], in_=velt)
            wenc_sb = cpool.tile([128, (NP // 128) * GSL], BF)
            for k in range(NP // 128):
                nc.sync.dma_start(out=wenc_sb[:, k * GSL:(k + 1) * GSL],
                                  in_=wenc[k * 128:(k + 1) * 128, :])
            pct_sb = cpool.tile([128, (NP // 128) * B], BF)
            for k in range(NP // 128):
                nc.sync.dma_start(out=pct_sb[:, k * B:(k + 1) * B],
                                  in_=pct[k * 128:(k + 1) * 128, :])

            def allgather(h_bf_tiles):
                """h slice tiles [128,B]x4 -> gathered hT sbuf [128, KT*B]."""
                ag_in = dram.tile([GSL, B], BF, tag="ag_in")
                ag_out = dram.tile([NG, B], BF, tag="ag_out")
                for m in range(MT):
                    nc.sync.dma_start(out=ag_in[m * 128:(m + 1) * 128, :],
                                      in_=h_bf_tiles[m][:])
                nc.gpsimd.collective_compute(
                    "AllGather", mybir.AluOpType.bypass,
                    replica_groups=[list(range(N_CORES))],
                    ins=[ag_in.opt()], outs=[ag_out.opt()],
                )
                hT = hpool.tile([128, KT * B], BF, tag="hT")
                for k in range(KT):
                    nc.sync.dma_start(out=hT[:, k * B:(k + 1) * B],
                                      in_=ag_out[k * 128:(k + 1) * 128, :])
                return hT

            # ---- encoder: h0 slice = W_enc[slice,:] @ pcT ----
            h0_bf = []
            for m in range(MT):
                p = ps.tile([128, B], F32, tag="ps")
                for k in range(NP // 128):
                    nc.tensor.matmul(
                        p[:], wenc_sb[:, k * GSL + m * 128: k * GSL + (m + 1) * 128],
                        pct_sb[:, k * B:(k + 1) * B],
                        start=(k == 0), stop=(k == NP // 128 - 1))
                hb = hbfpool.tile([128, B], BF, tag="hbf")
                nc.scalar.copy(hb[:], p[:])
                h0_bf.append(hb)
            hT = allgather(h0_bf)

            def decoder(hT_src, t):
                p = psd.tile([PSL, B], F32, tag="psd")
                for k in range(KT):
                    nc.tensor.matmul(
                        p[:], wdec_sb[:, k * PSL:(k + 1) * PSL],
                        hT_src[:, k * B:(k + 1) * B],
                        start=(k == 0), stop=(k == KT - 1))
                lsb = lpool.tile([PSL, B], F32, tag="lsb")
                nc.scalar.copy(lsb[:], p[:])
                nc.sync.dma_start(out=l_out[t, :, :], in_=lsb[:])

            # ---- recurrence ----
            for r in range(repeat):
                for t in range(seq):
                    psums = [ps.tile([128, B], F32, tag="ps", name=f"ps_{t}_{m}")
                             for m in range(MT)]
                    # velocity projection opens each accumulation (K=2)
                    for m in range(MT):
                        nc.tensor.matmul(
                            psums[m][:], wih_sb[:, m * 128:(m + 1) * 128],
                            velt_sb[:, t * B:(t + 1) * B],
                            start=True, stop=False)
                    # k-outer so matmuls gate on individual gathered chunks
                    for k in range(KT):
                        for m in range(MT):
                            nc.tensor.matmul(
                                psums[m][:],
                                whh_sb[:, k * GSL + m * 128: k * GSL + (m + 1) * 128],
                                hT[:, k * B:(k + 1) * B],
                                start=False, stop=(k == KT - 1))
                    h_bf = []
                    for m in range(MT):
                        gsb = gpool.tile([128, B], F32, tag="g")
                        nc.scalar.activation(gsb[:], psums[m][:],
                                             mybir.ActivationFunctionType.Relu)
                        hb = hbfpool.tile([128, B], BF, tag="hbf")
                        nc.vector.tensor_copy(hb[:], gsb[:])
                        nc.sync.dma_start(out=g_out[t, m * 128:(m + 1) * 128, :],
                                          in_=gsb[:])
                        h_bf.append(hb)
                    hT_next = allgather(h_bf)
                    # decoder for t-1 runs on hT (still gathered) while the
                    # AllGather for step t is in flight
                    if t >= 1:
                        decoder(hT, t - 1)
                    hT = hT_next
                decoder(hT, seq - 1)
    nc.compile()
    return nc


_NC_CACHE = {}


def kernel(velocity, init_pc, W_enc, W_ih, W_hh, W_dec):
    velocity = np.asarray(velocity, dtype=np.float32)
    init_pc = np.asarray(init_pc, dtype=np.float32)
    W_enc = np.asarray(W_enc, dtype=np.float32)
    W_ih = np.asarray(W_ih, dtype=np.float32)
    W_hh = np.asarray(W_hh, dtype=np.float32)
    W_dec = np.asarray(W_dec, dtype=np.float32)
    seq = velocity.shape[0]

    bf = ml_dtypes.bfloat16
    pct = np.ascontiguousarray(init_pc.T).astype(bf)              # [NP, B]
    velt = np.ascontiguousarray(
        velocity.transpose(2, 0, 1).reshape(2, seq * B)).astype(bf)
    whhT = np.ascontiguousarray(W_hh.T).astype(bf)                # [NG, NG]
    wencT = np.ascontiguousarray(W_enc.T).astype(bf)              # [NP, NG]
    wdecT = np.ascontiguousarray(W_dec.T).astype(bf)              # [NG, NP]
    wihT = np.ascontiguousarray(W_ih.T).astype(bf)                # [2, NG]

    in_maps = []
    for c in range(N_CORES):
        in_maps.append({
            "whh": np.ascontiguousarray(whhT[:, c * GSL:(c + 1) * GSL]),
            "wenc": np.ascontiguousarray(wencT[:, c * GSL:(c + 1) * GSL]),
            "wdec": np.ascontiguousarray(wdecT[:, c * PSL:(c + 1) * PSL]),
            "wih": np.ascontiguousarray(wihT[:, c * GSL:(c + 1) * GSL]),
            "pct": pct,
            "velt": velt,
        })

    key = (seq, 1)
    if key not in _NC_CACHE:
        _NC_CACHE[key] = build_nc(seq=seq)
    nc = _NC_CACHE[key]
    res = run_bass_kernel_spmd(nc, in_maps, core_ids=list(range(N_CORES)))

    g = np.concatenate([res.results[c]["g_out"] for c in range(N_CORES)],
                       axis=1)                  # [seq, NG, B]
    g = np.ascontiguousarray(g.transpose(0, 2, 1))                # [seq, B, NG]
    logits = np.concatenate([res.results[c]["l_out"] for c in range(N_CORES)],
                            axis=1)             # [seq, NP, B]
    logits = np.ascontiguousarray(logits.transpose(0, 2, 1))      # [seq, B, NP]
    return logits, g


# revision 4
# speedup vs baseline: 139.4379x; 139.4379x over previous
"""GridNetwork RNN kernel for 8 Trainium2 NeuronCores.

Strategy: tensor-shard W_hh by output rows (512 per core), full batch on
every core, per-step AllGather of the hidden state in bf16. W_hh stays
SBUF-resident (bf16, 4MB/core). The decoder for step t-1 runs while the
AllGather for step t is in flight, hiding collective latency behind PE
work. The velocity input projection is folded into each step's PSUM
accumulation as an extra K=2 matmul, so vx is never materialized.

Layouts are transposed ([feature, batch]) so the batch (200) is the
matmul free dim and features map to partitions/contraction.
"""
import sys
sys.path.insert(0, '/opt/trn_rl_repo')
import numpy as np
import ml_dtypes

import concourse.bass as bass
import concourse.bacc as bacc
import concourse.tile as tile
import concourse.mybir as mybir
from concourse.bass_utils import run_bass_kernel_spmd

SEQ, B, NP, NG = 100, 200, 512, 4096
N_CORES = 8
GSL = NG // N_CORES      # 512 g-rows per core
PSL = NP // N_CORES      # 64 logit rows per core
KT = NG // 128           # 32 K tiles over the hidden dim
MT = GSL // 128          # 4 M tiles per core's W_hh slice

BF = mybir.dt.bfloat16
F32 = mybir.dt.float32


def build_nc(seq=SEQ, repeat=1, timing=False):
    nc = bacc.Bacc("TRN2", target_bir_lowering=False, debug=False,
                   num_devices=N_CORES)
    whh = nc.dram_tensor("whh", [NG, GSL], BF, kind="ExternalInput").ap()
    wenc = nc.dram_tensor("wenc", [NP, GSL], BF, kind="ExternalInput").ap()
    wdec = nc.dram_tensor("wdec", [NG, PSL], BF, kind="ExternalInput").ap()
    wih = nc.dram_tensor("wih", [2, GSL], BF, kind="ExternalInput").ap()
    pct = nc.dram_tensor("pct", [NP, B], BF, kind="ExternalInput").ap()
    velt = nc.dram_tensor("velt", [2, seq * B], BF, kind="ExternalInput").ap()
    l_out = nc.dram_tensor("l_out", [seq, PSL, B], F32, kind="ExternalOutput").ap()

    with tile.TileContext(nc) as tc:
        with (
            tc.tile_pool(name="const", bufs=1) as cpool,
            tc.tile_pool(name="hbuf", bufs=3) as hpool,
            tc.tile_pool(name="hbf", bufs=4) as hbfpool,
            tc.tile_pool(name="lsb", bufs=4) as lpool,
            tc.tile_pool(name="ps", bufs=5, space="PSUM") as ps,
            tc.tile_pool(name="psd", bufs=2, space="PSUM") as psd,
            tc.tile_pool(name="dram", bufs=3, space="DRAM") as dram,
            tc.tile_pool(name="gdram", bufs=1, space="DRAM") as gdram,
        ):
            if timing:
                g_out = gdram.tile([seq, GSL, B], BF, name="g_int")
            else:
                g_out = nc.dram_tensor("g_out", [seq, GSL, B], BF,
                                       kind="ExternalOutput").ap()
            # resident weights
            whh_sb = cpool.tile([128, KT * GSL], BF)       # [:, k*512+m*128 ..]
            for k in range(0, KT, 4):
                nc.sync.dma_start(
                    out=whh_sb[:, k * GSL:(k + 4) * GSL].rearrange(
                        "p (a b) -> p a b", a=4),
                    in_=whh[k * 128:(k + 4) * 128, :].rearrange(
                        "(a p) b -> p a b", p=128))
            wdec_sb = cpool.tile([128, KT * PSL], BF)
            for k in range(0, KT, 8):
                nc.sync.dma_start(
                    out=wdec_sb[:, k * PSL:(k + 8) * PSL].rearrange(
                        "p (a b) -> p a b", a=8),
                    in_=wdec[k * 128:(k + 8) * 128, :].rearrange(
                        "(a p) b -> p a b", p=128))
            wih_sb = cpool.tile([2, GSL], BF)
            nc.sync.dma_start(out=wih_sb[:], in_=wih)
            velt_sb = cpool.tile([2, seq * B], BF)
            nc.sync.dma_start(out=velt_sb[:], in_=velt)
            wenc_sb = cpool.tile([128, (NP // 128) * GSL], BF)
            nc.sync.dma_start(
                out=wenc_sb[:].rearrange("p (a b) -> p a b", a=4),
                in_=wenc.rearrange("(a p) b -> p a b", p=128))
            pct_sb = cpool.tile([128, (NP // 128) * B], BF)
            nc.sync.dma_start(
                out=pct_sb[:].rearrange("p (a b) -> p a b", a=4),
                in_=pct.rearrange("(a p) b -> p a b", p=128))

            def allgather(hbf, t):
                """hbf [128, MT*B] bf16 -> gathered hT sbuf [128, KT*B]."""
                ag_in = dram.tile([GSL, B], BF, tag="ag_in",
                                  name=f"ag_in_{t}")
                ag_out = dram.tile([NG, B], BF, tag="ag_out",
                                   addr_space="Shared", name=f"ag_out_{t}")
                nc.sync.dma_start(
                    out=ag_in.rearrange("(a p) b -> p a b", p=128),
                    in_=hbf[:].rearrange("p (a b) -> p a b", a=MT))
                nc.gpsimd.collective_compute(
                    "AllGather", mybir.AluOpType.bypass,
                    replica_groups=[list(range(N_CORES))],
                    ins=[ag_in.opt()], outs=[ag_out.opt()],
                )
                hT = hpool.tile([128, KT * B], BF, tag="hT", name=f"hT_{t}")
                for k in range(0, KT, 4):
                    nc.sync.dma_start(
                        out=hT[:, k * B:(k + 4) * B].rearrange(
                            "p (a b) -> p a b", a=4),
                        in_=ag_out[k * 128:(k + 4) * 128, :].rearrange(
                            "(a p) b -> p a b", p=128))
                return hT

            # ---- encoder: h0 slice = W_enc[slice,:] @ pcT ----
            h0bf = hbfpool.tile([128, MT * B], BF, tag="hbf", name="h0bf")
            for m in range(MT):
                p = ps.tile([128, B], F32, tag="ps", name=f"ps_h0_{m}")
                for k in range(NP // 128):
                    nc.tensor.matmul(
                        p[:], wenc_sb[:, k * GSL + m * 128: k * GSL + (m + 1) * 128],
                        pct_sb[:, k * B:(k + 1) * B],
                        start=(k == 0), stop=(k == NP // 128 - 1))
                nc.scalar.copy(h0bf[:, m * B:(m + 1) * B], p[:])
            hT = allgather(h0bf, -1)

            def decoder(hT_src, t):
                p = psd.tile([PSL, B], F32, tag="psd", name=f"psd_{t}")
                for k in range(KT):
                    nc.tensor.matmul(
                        p[:], wdec_sb[:, k * PSL:(k + 1) * PSL],
                        hT_src[:, k * B:(k + 1) * B],
                        start=(k == 0), stop=(k == KT - 1))
                lsb = lpool.tile([PSL, B], F32, tag="lsb", name=f"lsb_{t}")
                nc.scalar.copy(lsb[:], p[:])
                nc.sync.dma_start(out=l_out[t, :, :], in_=lsb[:])

            # ---- recurrence ----
            for r in range(repeat):
                for t in range(seq):
                    psums = [ps.tile([128, B], F32, tag="ps", name=f"ps_{r}_{t}_{m}")
                             for m in range(MT)]
                    # velocity projection opens each accumulation (K=2)
                    for m in range(MT):
                        nc.tensor.matmul(
                            psums[m][:], wih_sb[:, m * 128:(m + 1) * 128],
                            velt_sb[:, t * B:(t + 1) * B],
                            start=True, stop=False)
                    # k-outer so matmuls gate on individual gathered chunks
                    for k in range(KT):
                        for m in range(MT):
                            nc.tensor.matmul(
                                psums[m][:],
                                whh_sb[:, k * GSL + m * 128: k * GSL + (m + 1) * 128],
                                hT[:, k * B:(k + 1) * B],
                                start=False, stop=(k == KT - 1))
                    hbf = hbfpool.tile([128, MT * B], BF, tag="hbf",
                                       name=f"hbf_{r}_{t}")
                    for m in range(MT):
                        nc.scalar.activation(hbf[:, m * B:(m + 1) * B], psums[m][:],
                                             mybir.ActivationFunctionType.Relu)
                    nc.sync.dma_start(
                        out=g_out[t].rearrange("(a p) b -> p a b", p=128),
                        in_=hbf[:].rearrange("p (a b) -> p a b", a=MT))
                    hT_next = allgather(hbf, t + r * seq)
                    # decoder for t-1 runs on hT (still gathered) while the
                    # AllGather for step t is in flight
                    if t >= 1:
                        decoder(hT, t - 1)
                    hT = hT_next
                decoder(hT, seq - 1)
    nc.compile()
    return nc


_NC_CACHE = {}


def kernel(velocity, init_pc, W_enc, W_ih, W_hh, W_dec):
    velocity = np.asarray(velocity, dtype=np.float32)
    init_pc = np.asarray(init_pc, dtype=np.float32)
    W_enc = np.asarray(W_enc, dtype=np.float32)
    W_ih = np.asarray(W_ih, dtype=np.float32)
    W_hh = np.asarray(W_hh, dtype=np.float32)
    W_dec = np.asarray(W_dec, dtype=np.float32)
    seq = velocity.shape[0]

    bf = ml_dtypes.bfloat16
    pct = np.ascontiguousarray(init_pc.T).astype(bf)              # [NP, B]
    velt = np.ascontiguousarray(
        velocity.transpose(2, 0, 1).reshape(2, seq * B)).astype(bf)
    whhT = np.ascontiguousarray(W_hh.T).astype(bf)                # [NG, NG]
    wencT = np.ascontiguousarray(W_enc.T).astype(bf)              # [NP, NG]
    wdecT = np.ascontiguousarray(W_dec.T).astype(bf)              # [NG, NP]
    wihT = np.ascontiguousarray(W_ih.T).astype(bf)                # [2, NG]

    in_maps = []
    for c in range(N_CORES):
        in_maps.append({
            "whh": np.ascontiguousarray(whhT[:, c * GSL:(c + 1) * GSL]),
            "wenc": np.ascontiguousarray(wencT[:, c * GSL:(c + 1) * GSL]),
            "wdec": np.ascontiguousarray(wdecT[:, c * PSL:(c + 1) * PSL]),
            "wih": np.ascontiguousarray(wihT[:, c * GSL:(c + 1) * GSL]),
            "pct": pct,
            "velt": velt,
        })

    key = (seq, 1)
    if key not in _NC_CACHE:
        _NC_CACHE[key] = build_nc(seq=seq)
    nc = _NC_CACHE[key]
    res = run_bass_kernel_spmd(nc, in_maps, core_ids=list(range(N_CORES)))

    g = np.concatenate([res.results[c]["g_out"].astype(np.float32)
                        for c in range(N_CORES)], axis=1)         # [seq, NG, B]
    g = np.ascontiguousarray(g.transpose(0, 2, 1))                # [seq, B, NG]
    logits = np.concatenate([res.results[c]["l_out"] for c in range(N_CORES)],
                            axis=1)             # [seq, NP, B]
    logits = np.ascontiguousarray(logits.transpose(0, 2, 1))      # [seq, B, NP]
    return logits, g


# revision 6
# speedup vs baseline: 273.4437x; 1.9610x over previous
"""GridNetwork RNN kernel for 8 Trainium2 NeuronCores.

Strategy: tensor-shard W_hh by output rows (512 per core), full batch on
every core, per-step AllGather of the hidden state in bf16. W_hh stays
SBUF-resident (bf16, 4MB/core). The decoder for step t-1 runs while the
AllGather for step t is in flight, hiding collective latency behind PE
work. The velocity input projection is folded into each step's PSUM
accumulation as an extra K=2 matmul, so vx is never materialized.

Layouts are transposed ([feature, batch]) so the batch (200) is the
matmul free dim and features map to partitions/contraction.
"""
import sys
sys.path.insert(0, '/opt/trn_rl_repo')
import numpy as np
import ml_dtypes

import concourse.bass as bass
import concourse.bacc as bacc
import concourse.tile as tile
import concourse.mybir as mybir
from concourse.bass_utils import run_bass_kernel_spmd

SEQ, B, NP, NG = 100, 200, 512, 4096
N_CORES = 8
GSL = NG // N_CORES      # 512 g-rows per core
PSL = NP // N_CORES      # 64 logit rows per core
KT = NG // 128           # 32 K tiles over the hidden dim
MT = GSL // 128          # 4 M tiles per core's W_hh slice

BF = mybir.dt.bfloat16
F32 = mybir.dt.float32


def build_nc(seq=SEQ, repeat=1, timing=False, ag_mode="real"):
    nc = bacc.Bacc("TRN2", target_bir_lowering=False, debug=False,
                   num_devices=N_CORES)
    whh = nc.dram_tensor("whh", [NG, GSL], BF, kind="ExternalInput").ap()
    wenc = nc.dram_tensor("wenc", [NP, GSL], BF, kind="ExternalInput").ap()
    wdec = nc.dram_tensor("wdec", [NG, PSL], BF, kind="ExternalInput").ap()
    wih = nc.dram_tensor("wih", [2, GSL], BF, kind="ExternalInput").ap()
    pct = nc.dram_tensor("pct", [NP, B], BF, kind="ExternalInput").ap()
    velt = nc.dram_tensor("velt", [2, seq * B], BF, kind="ExternalInput").ap()
    l_out = nc.dram_tensor("l_out", [seq, PSL, B], F32, kind="ExternalOutput").ap()

    with tile.TileContext(nc) as tc:
        with (
            tc.tile_pool(name="const", bufs=1) as cpool,
            tc.tile_pool(name="hbuf", bufs=3) as hpool,
            tc.tile_pool(name="hbf", bufs=4) as hbfpool,
            tc.tile_pool(name="lsb", bufs=4) as lpool,
            tc.tile_pool(name="ps", bufs=5, space="PSUM") as ps,
            tc.tile_pool(name="psd", bufs=2, space="PSUM") as psd,
            tc.tile_pool(name="dram", bufs=3, space="DRAM") as dram,
            tc.tile_pool(name="gdram", bufs=1, space="DRAM") as gdram,
        ):
            if timing:
                g_out = gdram.tile([seq, GSL, B], BF, name="g_int")
            else:
                g_out = nc.dram_tensor("g_out", [seq, GSL, B], BF,
                                       kind="ExternalOutput").ap()
            # resident weights
            whh_sb = cpool.tile([128, KT * GSL], BF)       # [:, k*512+m*128 ..]
            for k in range(0, KT, 4):
                nc.sync.dma_start(
                    out=whh_sb[:, k * GSL:(k + 4) * GSL].rearrange(
                        "p (a b) -> p a b", a=4),
                    in_=whh[k * 128:(k + 4) * 128, :].rearrange(
                        "(a p) b -> p a b", p=128))
            wdec_sb = cpool.tile([128, KT * PSL], BF)
            for k in range(0, KT, 8):
                nc.sync.dma_start(
                    out=wdec_sb[:, k * PSL:(k + 8) * PSL].rearrange(
                        "p (a b) -> p a b", a=8),
                    in_=wdec[k * 128:(k + 8) * 128, :].rearrange(
                        "(a p) b -> p a b", p=128))
            wih_sb = cpool.tile([2, GSL], BF)
            nc.sync.dma_start(out=wih_sb[:], in_=wih)
            velt_sb = cpool.tile([2, seq * B], BF)
            nc.sync.dma_start(out=velt_sb[:], in_=velt)
            wenc_sb = cpool.tile([128, (NP // 128) * GSL], BF)
            nc.sync.dma_start(
                out=wenc_sb[:].rearrange("p (a b) -> p a b", a=4),
                in_=wenc.rearrange("(a p) b -> p a b", p=128))
            pct_sb = cpool.tile([128, (NP // 128) * B], BF)
            nc.sync.dma_start(
                out=pct_sb[:].rearrange("p (a b) -> p a b", a=4),
                in_=pct.rearrange("(a p) b -> p a b", p=128))

            def allgather(hbf, t):
                """hbf [128, MT*B] bf16 -> gathered hT sbuf [128, KT*B]."""
                ag_in = dram.tile([GSL, B], BF, tag="ag_in",
                                  name=f"ag_in_{t}")
                ag_out = dram.tile([NG, B], BF, tag="ag_out",
                                   addr_space="Shared", name=f"ag_out_{t}")
                nc.sync.dma_start(
                    out=ag_in.rearrange("(a p) b -> p a b", p=128),
                    in_=hbf[:].rearrange("p (a b) -> p a b", a=MT))
                if ag_mode == "real":
                    nc.gpsimd.collective_compute(
                        "AllGather", mybir.AluOpType.bypass,
                        replica_groups=[list(range(N_CORES))],
                        ins=[ag_in.opt()], outs=[ag_out.opt()],
                    )
                    src = ag_out
                else:  # timing experiment: same DMA chain, no collective
                    src = None
                hT = hpool.tile([128, KT * B], BF, tag="hT", name=f"hT_{t}")
                for k in range(0, KT, 4):
                    src_ap = (ag_out[k * 128:(k + 4) * 128, :] if src is not None
                              else ag_in[:, :])
                    nc.sync.dma_start(
                        out=hT[:, k * B:(k + 4) * B].rearrange(
                            "p (a b) -> p a b", a=4),
                        in_=src_ap.rearrange("(a p) b -> p a b", p=128))
                return hT

            # ---- encoder: h0 slice = W_enc[slice,:] @ pcT ----
            h0bf = hbfpool.tile([128, MT * B], BF, tag="hbf", name="h0bf")
            for m in range(MT):
                p = ps.tile([128, B], F32, tag="ps", name=f"ps_h0_{m}")
                for k in range(NP // 128):
                    nc.tensor.matmul(
                        p[:], wenc_sb[:, k * GSL + m * 128: k * GSL + (m + 1) * 128],
                        pct_sb[:, k * B:(k + 1) * B],
                        start=(k == 0), stop=(k == NP // 128 - 1))
                nc.scalar.copy(h0bf[:, m * B:(m + 1) * B], p[:])
            hT = allgather(h0bf, -1)

            def decoder(hT_src, t):
                p = psd.tile([PSL, B], F32, tag="psd", name=f"psd_{t}")
                for k in range(KT):
                    nc.tensor.matmul(
                        p[:], wdec_sb[:, k * PSL:(k + 1) * PSL],
                        hT_src[:, k * B:(k + 1) * B],
                        start=(k == 0), stop=(k == KT - 1))
                lsb = lpool.tile([PSL, B], F32, tag="lsb", name=f"lsb_{t}")
                nc.scalar.copy(lsb[:], p[:])
                nc.sync.dma_start(out=l_out[t, :, :], in_=lsb[:])

            # ---- recurrence ----
            for r in range(repeat):
                for t in range(seq):
                    psums = [ps.tile([128, B], F32, tag="ps", name=f"ps_{r}_{t}_{m}")
                             for m in range(MT)]
                    # velocity projection opens each accumulation (K=2)
                    for m in range(MT):
                        nc.tensor.matmul(
                            psums[m][:], wih_sb[:, m * 128:(m + 1) * 128],
                            velt_sb[:, t * B:(t + 1) * B],
                            start=True, stop=False)
                    # k-outer so matmuls gate on individual gathered chunks
                    for k in range(KT):
                        for m in range(MT):
                            nc.tensor.matmul(
                                psums[m][:],
                                whh_sb[:, k * GSL + m * 128: k * GSL + (m + 1) * 128],
                                hT[:, k * B:(k + 1) * B],
                                start=False, stop=(k == KT - 1))
                    hbf = hbfpool.tile([128, MT * B], BF, tag="hbf",
                                       name=f"hbf_{r}_{t}")
                    for m in range(MT):
                        nc.scalar.activation(hbf[:, m * B:(m + 1) * B], psums[m][:],
                                             mybir.ActivationFunctionType.Relu)
                    nc.sync.dma_start(
                        out=g_out[t].rearrange("(a p) b -> p a b", p=128),
                        in_=hbf[:].rearrange("p (a b) -> p a b", a=MT))
                    hT_next = allgather(hbf, t + r * seq)
                    # decoder for t-1 runs on hT (still gathered) while the
                    # AllGather for step t is in flight
                    if t >= 1:
                        decoder(hT, t - 1)
                    hT = hT_next
                decoder(hT, seq - 1)
    nc.compile()
    return nc


_NC_CACHE = {}


def kernel(velocity, init_pc, W_enc, W_ih, W_hh, W_dec):
    velocity = np.asarray(velocity, dtype=np.float32)
    init_pc = np.asarray(init_pc, dtype=np.float32)
    W_enc = np.asarray(W_enc, dtype=np.float32)
    W_ih = np.asarray(W_ih, dtype=np.float32)
    W_hh = np.asarray(W_hh, dtype=np.float32)
    W_dec = np.asarray(W_dec, dtype=np.float32)
    seq = velocity.shape[0]

    bf = ml_dtypes.bfloat16
    pct = np.ascontiguousarray(init_pc.T).astype(bf)              # [NP, B]
    velt = np.ascontiguousarray(
        velocity.transpose(2, 0, 1).reshape(2, seq * B)).astype(bf)
    whhT = np.ascontiguousarray(W_hh.T).astype(bf)                # [NG, NG]
    wencT = np.ascontiguousarray(W_enc.T).astype(bf)              # [NP, NG]
    wdecT = np.ascontiguousarray(W_dec.T).astype(bf)              # [NG, NP]
    wihT = np.ascontiguousarray(W_ih.T).astype(bf)                # [2, NG]

    in_maps = []
    for c in range(N_CORES):
        in_maps.append({
            "whh": np.ascontiguousarray(whhT[:, c * GSL:(c + 1) * GSL]),
            "wenc": np.ascontiguousarray(wencT[:, c * GSL:(c + 1) * GSL]),
            "wdec": np.ascontiguousarray(wdecT[:, c * PSL:(c + 1) * PSL]),
            "wih": np.ascontiguousarray(wihT[:, c * GSL:(c + 1) * GSL]),
            "pct": pct,
            "velt": velt,
        })

    key = (seq, 1)
    if key not in _NC_CACHE:
        _NC_CACHE[key] = build_nc(seq=seq)
    nc = _NC_CACHE[key]
    res = run_bass_kernel_spmd(nc, in_maps, core_ids=list(range(N_CORES)))

    g = np.concatenate([res.results[c]["g_out"].astype(np.float32)
                        for c in range(N_CORES)], axis=1)         # [seq, NG, B]
    g = np.ascontiguousarray(g.transpose(0, 2, 1))                # [seq, B, NG]
    logits = np.concatenate([res.results[c]["l_out"] for c in range(N_CORES)],
                            axis=1)             # [seq, NP, B]
    logits = np.ascontiguousarray(logits.transpose(0, 2, 1))      # [seq, B, NP]
    return logits, g
